# revision 1
# baseline (speedup 1.0000x reference)
"""AttentionPooling1D Trainium2 kernel.

Reference computation (per batch element b):
    scores[s] = x[b, s, :] @ w[0]                  # [S]
    scores    = where(mask[b] != 0, scores, -inf)
    probs     = softmax(scores)                    # [S]
    out[b, :] = probs @ x[b, :, :]                 # [D]

Strategy (memory-bound, one pass over x):
  - Shard batch B=64 across 8 cores (8 per core), no communication.
  - Per core, stream x in [128 s-rows, D] chunks (super_ chunks per DMA).
  - VectorE multiplies each chunk by the broadcast w (y = x * w_rep);
    ScalarE activation(Copy, accum_out=...) reduces y along the free dim
    to the per-row dot products (scores), one column per chunk.
  - Mask handling: scores += bias where bias is 0 (keep) or -30000 (mask)
    -> exp underflows to exactly 0, so masked rows drop out of both the
    softmax numerator and denominator; no -inf or max-subtraction needed
    (scores ~ N(0,1), exp cannot overflow). One batched ScalarE exp per
    superchunk.
  - TensorE accumulates numerator acc[1, D] += e^T @ x_chunk and the
    denominator l += e^T @ ones in PSUM across all 32 chunks of a batch
    (float32r: fp32 in/out, FP22 multiply, fp32 accumulate, 1 cycle/row).
  - Final: out[b] = acc * (1/l) via VectorE, DMA to DRAM.

Host-side prep (negligible bytes): broadcast w to [128, D]; turn the int32
mask into the additive f32 bias laid out as [128, batch*chunk] columns.
"""

import numpy as np

B, S, D = 64, 4096, 1024
N_CORES = 8
B_PC = B // N_CORES      # batches per core
P = 128                  # SBUF partitions
NEG_BIAS = -30000.0      # exp(x + NEG_BIAS) == 0.0 in fp32 for any plausible x


def build_bass(b_pc=B_PC, s=S, d=D, super_=2, x_bufs=6, use_bf16=False):
    """Build the single-core Bass program. Parameterized so tests can build
    a small config for CoreSim."""
    import concourse.bacc as bacc
    import concourse.tile as tile
    from concourse import mybir

    cpb = s // P             # chunks per batch
    scpb = cpb // super_     # superchunks per batch
    assert scpb * super_ == cpb and cpb * P == s
    assert d % 1024 == 0 or d <= 1024

    f32 = mybir.dt.float32
    f32r = mybir.dt.float32r
    bf16 = mybir.dt.bfloat16
    # xd: dtype of the streamed x / w / e operands; mm views feed the PE
    xd = bf16 if use_bf16 else f32

    nc = bacc.Bacc(trn_type="TRN2", target_bir_lowering=False, debug=False)
    x_d = nc.declare_dram_parameter("x", [b_pc, s, d], f32, isOutput=False)
    w_d = nc.declare_dram_parameter("w_rep", [P, d], xd, isOutput=False)
    bias_d = nc.declare_dram_parameter("bias", [P, b_pc * cpb], f32, isOutput=False)
    ones_d = nc.declare_dram_parameter("ones", [P, 2], xd, isOutput=False)
    out_d = nc.declare_dram_parameter("out", [b_pc, d], f32, isOutput=True)

    def mm(ap):
        # PE-view of an operand: fp32 operands must be fed as float32r
        # (FP22-truncate-on-read) to stream at 1 cycle/row; bf16 is native.
        return ap if use_bf16 else ap.bitcast(f32r)

    n_half = d // 2          # 512 for the real problem (PSUM fp32 matmul max)
    assert n_half <= 512

    with tile.TileContext(nc) as tc:
        with (
            tc.tile_pool(name="xpool", bufs=x_bufs) as xpool,
            tc.tile_pool(name="ypool", bufs=3) as ypool,
            tc.tile_pool(name="consts", bufs=1) as consts,
            tc.tile_pool(name="small", bufs=8) as small,
            tc.tile_pool(name="outp", bufs=2) as outp,
            tc.tile_pool(name="psum", bufs=2, space="PSUM") as psum_pool,
        ):
            w_sb = consts.tile([P, d], xd)
            nc.sync.dma_start(out=w_sb, in_=w_d[:])
            bias_sb = consts.tile([P, b_pc * cpb], f32)
            nc.sync.dma_start(out=bias_sb, in_=bias_d[:])
            ones_sb = consts.tile([P, 2], xd)
            nc.sync.dma_start(out=mm(ones_sb), in_=mm(ones_d[:]))

            for b in range(b_pc):
                acc0 = psum_pool.tile([1, n_half], f32, tag="acc0")
                acc1 = psum_pool.tile([1, n_half], f32, tag="acc1")
                lps = psum_pool.tile([1, 2], f32, tag="l")
                for sc in range(scpb):
                    xt = xpool.tile([P, super_, d], xd, tag="xt")
                    src = x_d[b, sc * super_ * P : (sc + 1) * super_ * P, :].rearrange(
                        "(j p) d -> p j d", p=P
                    )
                    if use_bf16:
                        # SWDGE casts fp32 -> bf16 inline; HBM read traffic
                        # is unchanged, SBUF tile halves, and the DVE
                        # multiply gets the 2x bf16 perf mode.
                        nc.gpsimd.dma_start(out=xt, in_=src)
                    else:
                        # Write through an f32r-typed AP: the fp32r matmuls
                        # below require their producer to emit fp32r (PE
                        # truncates to FP22 on read; bits are plain fp32).
                        nc.sync.dma_start(out=mm(xt), in_=mm(src))
                    # scores for all super_ chunks of this superchunk,
                    # one column each; exp'd in a single ACT op.
                    scores = small.tile([P, super_], f32, tag="scores")
                    for j in range(super_):
                        y = ypool.tile([P, d], xd, tag="y")
                        nc.vector.tensor_mul(y, xt[:, j, :], w_sb)
                        nc.scalar.activation(
                            y,
                            y,
                            mybir.ActivationFunctionType.Copy,
                            accum_out=scores[:, j : j + 1],
                        )
                    col0 = b * cpb + sc * super_
                    nc.vector.tensor_add(
                        scores, scores, bias_sb[:, col0 : col0 + super_]
                    )
                    e = small.tile([P, super_], xd, tag="e")
                    er = mm(e)
                    nc.scalar.activation(
                        er, scores, mybir.ActivationFunctionType.Exp
                    )
                    for j in range(super_):
                        c = sc * super_ + j
                        first = c == 0
                        last = c == cpb - 1
                        ej = er[:, j : j + 1]
                        nc.tensor.matmul(
                            acc0,
                            ej,
                            mm(xt[:, j, :n_half]),
                            start=first,
                            stop=last,
                        )
                        nc.tensor.matmul(
                            acc1,
                            ej,
                            mm(xt[:, j, n_half:]),
                            start=first,
                            stop=last,
                        )
                        nc.tensor.matmul(
                            lps,
                            ej,
                            mm(ones_sb),
                            start=first,
                            stop=last,
                        )
                linv = small.tile([1, 1], f32, tag="linv")
                nc.vector.reciprocal(linv, lps[:, 0:1])
                ob = outp.tile([1, d], f32, tag="ob")
                nc.vector.tensor_scalar_mul(ob[:, :n_half], acc0, linv)
                nc.vector.tensor_scalar_mul(ob[:, n_half:], acc1, linv)
                nc.sync.dma_start(out=out_d[b : b + 1, :], in_=ob)
    nc.compile()
    return nc


def make_in_maps(x, padding_mask, w, b_pc=B_PC, s=S, d=D, n_cores=N_CORES,
                 use_bf16=False):
    """Shard inputs and build per-core host-side tensors."""
    x = np.asarray(x, dtype=np.float32)
    padding_mask = np.asarray(padding_mask)
    w = np.asarray(w, dtype=np.float32)
    cpb = s // P
    bias = np.where(padding_mask != 0, np.float32(0.0), np.float32(NEG_BIAS))
    bias = bias.astype(np.float32)
    w_rep = np.ascontiguousarray(np.broadcast_to(w.reshape(1, d), (P, d)))
    if use_bf16:
        import ml_dtypes
        w_rep = w_rep.astype(ml_dtypes.bfloat16)
    in_maps = []
    for core in range(n_cores):
        xc = np.ascontiguousarray(x[core * b_pc : (core + 1) * b_pc])
        bc = bias[core * b_pc : (core + 1) * b_pc]  # [b_pc, s]
        # bias_sb[p, b*cpb + c] = bias for row s = c*128 + p of batch b
        bc = np.ascontiguousarray(
            bc.reshape(b_pc, cpb, P).transpose(2, 0, 1).reshape(P, b_pc * cpb)
        )
        ones = np.ones((P, 2), dtype=np.float32)
        if use_bf16:
            import ml_dtypes
            ones = ones.astype(ml_dtypes.bfloat16)
        in_maps.append({"x": xc, "w_rep": w_rep, "bias": bc, "ones": ones})
    return in_maps


_NC_CACHE = {}


def _get_nc():
    if "nc" not in _NC_CACHE:
        _NC_CACHE["nc"] = build_bass()
    return _NC_CACHE["nc"]


def kernel(x, padding_mask, w):
    from concourse.bass_utils import run_bass_kernel_spmd

    nc = _get_nc()
    in_maps = make_in_maps(x, padding_mask, w)
    res = run_bass_kernel_spmd(nc, in_maps, list(range(N_CORES)))
    outs = [res.results[c]["out"] for c in range(N_CORES)]
    return np.concatenate(outs, axis=0).astype(np.float32)


# ---------------------------------------------------------------------------
# Mask-gather variant: only unmasked rows are loaded (masked rows contribute
# exactly 0 to softmax numerator and denominator). Row indices come from the
# runtime mask (host-computed, passed as an int16 input) via dma_gather.
# ---------------------------------------------------------------------------

def build_bass_gather(b_pc=B_PC, s=S, d=D, cap_chunks=20, half_chunks=10,
                      x_bufs=4):
    import concourse.bacc as bacc
    import concourse.tile as tile
    from concourse import mybir

    cap = cap_chunks * P
    halves = cap_chunks // half_chunks
    assert halves * half_chunks == cap_chunks
    f32 = mybir.dt.float32
    f32r = mybir.dt.float32r
    i16 = mybir.dt.int16
    n_half = d // 2
    nidx_half = half_chunks * P           # rows per gather call

    nc = bacc.Bacc(trn_type="TRN2", target_bir_lowering=False, debug=False)
    x_d = nc.declare_dram_parameter("x", [b_pc, s, d], f32, isOutput=False)
    w_d = nc.declare_dram_parameter("w_rep", [P, d], f32, isOutput=False)
    bias_d = nc.declare_dram_parameter("bias", [P, b_pc * cap_chunks], f32,
                                       isOutput=False)
    idx_d = nc.declare_dram_parameter(
        "idx", [P, b_pc * halves * (nidx_half // 16)], i16, isOutput=False)
    ones_d = nc.declare_dram_parameter("ones", [P, 2], f32, isOutput=False)
    out_d = nc.declare_dram_parameter("out", [b_pc, d], f32, isOutput=True)

    with tile.TileContext(nc) as tc:
        with (
            tc.tile_pool(name="xpool", bufs=x_bufs) as xpool,
            tc.tile_pool(name="ypool", bufs=3) as ypool,
            tc.tile_pool(name="consts", bufs=1) as consts,
            tc.tile_pool(name="small", bufs=8) as small,
            tc.tile_pool(name="outp", bufs=2) as outp,
            tc.tile_pool(name="psum", bufs=2, space="PSUM") as psum_pool,
        ):
            w_sb = consts.tile([P, d], f32)
            nc.sync.dma_start(out=w_sb, in_=w_d[:])
            bias_sb = consts.tile([P, b_pc * cap_chunks], f32)
            nc.sync.dma_start(out=bias_sb, in_=bias_d[:])
            idx_sb = consts.tile([P, b_pc * halves * (nidx_half // 16)], i16)
            nc.sync.dma_start(out=idx_sb, in_=idx_d[:])
            ones_sb = consts.tile([P, 2], f32)
            nc.sync.dma_start(out=ones_sb.bitcast(f32r), in_=ones_d[:].bitcast(f32r))

            icols = nidx_half // 16
            for b in range(b_pc):
                acc0 = psum_pool.tile([1, n_half], f32, tag="acc0")
                acc1 = psum_pool.tile([1, n_half], f32, tag="acc1")
                lps = psum_pool.tile([1, 2], f32, tag="l")
                for h in range(halves):
                    xt = xpool.tile([P, half_chunks, d], f32, tag="xt")
                    islice = idx_sb[:, (b * halves + h) * icols
                                    : (b * halves + h + 1) * icols]
                    nc.gpsimd.dma_gather(
                        out_ap=xt.bitcast(f32r),
                        in_ap=x_d[b].bitcast(f32r),
                        idxs_ap=islice,
                        num_idxs=nidx_half,
                        num_idxs_reg=nidx_half,
                        elem_size=d,
                    )
                    scores = small.tile([P, half_chunks], f32, tag="scores")
                    for j in range(half_chunks):
                        y = ypool.tile([P, d], f32, tag="y")
                        nc.vector.tensor_mul(y, xt[:, j, :], w_sb)
                        nc.scalar.activation(
                            y, y, mybir.ActivationFunctionType.Copy,
                            accum_out=scores[:, j : j + 1],
                        )
                    col0 = b * cap_chunks + h * half_chunks
                    nc.vector.tensor_add(
                        scores, scores, bias_sb[:, col0 : col0 + half_chunks]
                    )
                    e = small.tile([P, half_chunks], f32, tag="e")
                    er = e.bitcast(f32r)
                    nc.scalar.activation(
                        er, scores, mybir.ActivationFunctionType.Exp
                    )
                    for j in range(half_chunks):
                        c = h * half_chunks + j
                        first = c == 0
                        last = c == cap_chunks - 1
                        ej = er[:, j : j + 1]
                        nc.tensor.matmul(acc0, ej, xt[:, j, :n_half].bitcast(f32r),
                                         start=first, stop=last)
                        nc.tensor.matmul(acc1, ej, xt[:, j, n_half:].bitcast(f32r),
                                         start=first, stop=last)
                        nc.tensor.matmul(lps, ej, ones_sb.bitcast(f32r),
                                         start=first, stop=last)
                linv = small.tile([1, 1], f32, tag="linv")
                nc.vector.reciprocal(linv, lps[:, 0:1])
                ob = outp.tile([1, d], f32, tag="ob")
                nc.vector.tensor_scalar_mul(ob[:, :n_half], acc0, linv)
                nc.vector.tensor_scalar_mul(ob[:, n_half:], acc1, linv)
                nc.sync.dma_start(out=out_d[b : b + 1, :], in_=ob)
    nc.compile()
    return nc


def make_in_maps_gather(x, padding_mask, w, b_pc=B_PC, s=S, d=D,
                        n_cores=N_CORES, cap_chunks=20, half_chunks=10):
    """Host prep for the gather variant. Returns None if any batch has more
    unmasked rows than cap_chunks*128 (caller falls back to dense)."""
    x = np.asarray(x, dtype=np.float32)
    padding_mask = np.asarray(padding_mask)
    w = np.asarray(w, dtype=np.float32)
    cap = cap_chunks * P
    halves = cap_chunks // half_chunks
    nidx_half = half_chunks * P
    icols = nidx_half // 16
    w_rep = np.ascontiguousarray(np.broadcast_to(w.reshape(1, d), (P, d)))
    in_maps = []
    for core in range(n_cores):
        xc = np.ascontiguousarray(x[core * b_pc : (core + 1) * b_pc])
        mc = padding_mask[core * b_pc : (core + 1) * b_pc]
        bias_cols = np.zeros((P, b_pc * cap_chunks), dtype=np.float32)
        idx_cols = np.zeros((16, b_pc * halves * icols), dtype=np.int16)
        for b in range(b_pc):
            keep = np.where(mc[b] != 0)[0]
            if len(keep) > cap:
                return None
            idxs = np.zeros(cap, dtype=np.int16)
            idxs[: len(keep)] = keep.astype(np.int16)
            biasvec = np.zeros(cap, dtype=np.float32)
            biasvec[len(keep):] = NEG_BIAS
            bias_cols[:, b * cap_chunks : (b + 1) * cap_chunks] = (
                biasvec.reshape(cap_chunks, P).T
            )
            for h in range(halves):
                part = idxs[h * nidx_half : (h + 1) * nidx_half]
                # index k -> partition k%16, column k//16
                idx_cols[:, (b * halves + h) * icols
                         : (b * halves + h + 1) * icols] = (
                    part.reshape(icols, 16).T
                )
        idx_full = np.ascontiguousarray(np.tile(idx_cols, (8, 1)))
        ones = np.ones((P, 2), dtype=np.float32)
        in_maps.append({
            "x": xc, "w_rep": w_rep, "bias": np.ascontiguousarray(bias_cols),
            "idx": idx_full, "ones": ones,
        })
    return in_maps


def build_bass_gather2(b_pc=B_PC, s=S, d=D, cap_chunks=20, x_bufs=6):
    """Mask-gather via per-chunk indirect_dma_start (plain InstDMACopy with
    dynamic AP — no GpSimd library overlay, unlike dma_gather)."""
    import concourse.bacc as bacc
    import concourse.bass as bass
    import concourse.tile as tile
    from concourse import mybir

    f32 = mybir.dt.float32
    f32r = mybir.dt.float32r
    i32 = mybir.dt.int32
    n_half = d // 2

    nc = bacc.Bacc(trn_type="TRN2", target_bir_lowering=False, debug=False)
    x_d = nc.declare_dram_parameter("x", [b_pc, s, d], f32, isOutput=False)
    w_d = nc.declare_dram_parameter("w_rep", [P, d], f32, isOutput=False)
    bias_d = nc.declare_dram_parameter("bias", [P, b_pc * cap_chunks], f32,
                                       isOutput=False)
    idx_d = nc.declare_dram_parameter("idx", [P, b_pc * cap_chunks], i32,
                                      isOutput=False)
    ones_d = nc.declare_dram_parameter("ones", [P, 2], f32, isOutput=False)
    out_d = nc.declare_dram_parameter("out", [b_pc, d], f32, isOutput=True)

    x_flat = x_d[:].rearrange("b s d -> (b s) d").bitcast(f32r)
    with tile.TileContext(nc) as tc:
        with (
            tc.tile_pool(name="xpool", bufs=x_bufs) as xpool,
            tc.tile_pool(name="ypool", bufs=3) as ypool,
            tc.tile_pool(name="consts", bufs=1) as consts,
            tc.tile_pool(name="small", bufs=8) as small,
            tc.tile_pool(name="outp", bufs=2) as outp,
            tc.tile_pool(name="psum", bufs=2, space="PSUM") as psum_pool,
        ):
            w_sb = consts.tile([P, d], f32)
            nc.sync.dma_start(out=w_sb, in_=w_d[:])
            bias_sb = consts.tile([P, b_pc * cap_chunks], f32)
            nc.sync.dma_start(out=bias_sb, in_=bias_d[:])
            idx_sb = consts.tile([P, b_pc * cap_chunks], i32)
            nc.sync.dma_start(out=idx_sb, in_=idx_d[:])
            ones_sb = consts.tile([P, 2], f32)
            nc.sync.dma_start(out=ones_sb.bitcast(f32r), in_=ones_d[:].bitcast(f32r))

            for b in range(b_pc):
                acc0 = psum_pool.tile([1, n_half], f32, tag="acc0")
                acc1 = psum_pool.tile([1, n_half], f32, tag="acc1")
                lps = psum_pool.tile([1, 2], f32, tag="l")
                for c in range(cap_chunks):
                    col = b * cap_chunks + c
                    xt = xpool.tile([P, d], f32, tag="xt")
                    nc.gpsimd.indirect_dma_start(
                        out=xt.bitcast(f32r),
                        out_offset=None,
                        in_=x_flat,
                        in_offset=bass.IndirectOffsetOnAxis(
                            ap=idx_sb[:, col : col + 1], axis=0
                        ),
                    )
                    y = ypool.tile([P, d], f32, tag="y")
                    nc.vector.tensor_mul(y, xt, w_sb)
                    scores = small.tile([P, 1], f32, tag="scores")
                    nc.scalar.activation(
                        y, y, mybir.ActivationFunctionType.Copy,
                        accum_out=scores,
                    )
                    e = small.tile([P, 1], f32, tag="e")
                    er = e.bitcast(f32r)
                    nc.scalar.activation(
                        er, scores, mybir.ActivationFunctionType.Exp,
                        bias=bias_sb[:, col : col + 1],
                    )
                    first = c == 0
                    last = c == cap_chunks - 1
                    nc.tensor.matmul(acc0, er, xt[:, :n_half].bitcast(f32r),
                                     start=first, stop=last)
                    nc.tensor.matmul(acc1, er, xt[:, n_half:].bitcast(f32r),
                                     start=first, stop=last)
                    nc.tensor.matmul(lps, er, ones_sb.bitcast(f32r),
                                     start=first, stop=last)
                linv = small.tile([1, 1], f32, tag="linv")
                nc.vector.reciprocal(linv, lps[:, 0:1])
                ob = outp.tile([1, d], f32, tag="ob")
                nc.vector.tensor_scalar_mul(ob[:, :n_half], acc0, linv)
                nc.vector.tensor_scalar_mul(ob[:, n_half:], acc1, linv)
                nc.sync.dma_start(out=out_d[b : b + 1, :], in_=ob)
    nc.compile()
    return nc


def make_in_maps_gather2(x, padding_mask, w, b_pc=B_PC, s=S, d=D,
                         n_cores=N_CORES, cap_chunks=20):
    x = np.asarray(x, dtype=np.float32)
    padding_mask = np.asarray(padding_mask)
    w = np.asarray(w, dtype=np.float32)
    cap = cap_chunks * P
    w_rep = np.ascontiguousarray(np.broadcast_to(w.reshape(1, d), (P, d)))
    in_maps = []
    for core in range(n_cores):
        xc = np.ascontiguousarray(x[core * b_pc : (core + 1) * b_pc])
        mc = padding_mask[core * b_pc : (core + 1) * b_pc]
        bias_cols = np.zeros((P, b_pc * cap_chunks), dtype=np.float32)
        idx_cols = np.zeros((P, b_pc * cap_chunks), dtype=np.int32)
        for b in range(b_pc):
            keep = np.where(mc[b] != 0)[0]
            if len(keep) > cap:
                return None
            idxs = np.full(cap, b * s, dtype=np.int32)
            idxs[: len(keep)] = keep + b * s
            biasvec = np.zeros(cap, dtype=np.float32)
            biasvec[len(keep):] = NEG_BIAS
            sl = slice(b * cap_chunks, (b + 1) * cap_chunks)
            bias_cols[:, sl] = biasvec.reshape(cap_chunks, P).T
            idx_cols[:, sl] = idxs.reshape(cap_chunks, P).T
        in_maps.append({
            "x": xc, "w_rep": w_rep,
            "bias": np.ascontiguousarray(bias_cols),
            "idx": np.ascontiguousarray(idx_cols),
            "ones": np.ones((P, 2), dtype=np.float32),
        })
    return in_maps



# revision 2
# speedup vs baseline: 2.1231x; 2.1231x over previous
"""AttentionPooling1D Trainium2 kernel.

Reference computation (per batch element b):
    scores[s] = x[b, s, :] @ w[0]                  # [S]
    scores    = where(mask[b] != 0, scores, -inf)
    probs     = softmax(scores)                    # [S]
    out[b, :] = probs @ x[b, :, :]                 # [D]

Strategy (memory-bound, one pass over x):
  - Shard batch B=64 across 8 cores (8 per core), no communication.
  - Per core, stream x in [128 s-rows, D] chunks (super_ chunks per DMA).
  - VectorE multiplies each chunk by the broadcast w (y = x * w_rep);
    ScalarE activation(Copy, accum_out=...) reduces y along the free dim
    to the per-row dot products (scores), one column per chunk.
  - Mask handling: scores += bias where bias is 0 (keep) or -30000 (mask)
    -> exp underflows to exactly 0, so masked rows drop out of both the
    softmax numerator and denominator; no -inf or max-subtraction needed
    (scores ~ N(0,1), exp cannot overflow). One batched ScalarE exp per
    superchunk.
  - TensorE accumulates numerator acc[1, D] += e^T @ x_chunk and the
    denominator l += e^T @ ones in PSUM across all 32 chunks of a batch
    (float32r: fp32 in/out, FP22 multiply, fp32 accumulate, 1 cycle/row).
  - Final: out[b] = acc * (1/l) via VectorE, DMA to DRAM.

Host-side prep (negligible bytes): broadcast w to [128, D]; turn the int32
mask into the additive f32 bias laid out as [128, batch*chunk] columns.
"""

import numpy as np

B, S, D = 64, 4096, 1024
N_CORES = 8
B_PC = B // N_CORES      # batches per core
P = 128                  # SBUF partitions
NEG_BIAS = -30000.0      # exp(x + NEG_BIAS) == 0.0 in fp32 for any plausible x


def build_bass(b_pc=B_PC, s=S, d=D, super_=2, x_bufs=6, use_bf16=False):
    """Build the single-core Bass program. Parameterized so tests can build
    a small config for CoreSim."""
    import concourse.bacc as bacc
    import concourse.tile as tile
    from concourse import mybir

    cpb = s // P             # chunks per batch
    scpb = cpb // super_     # superchunks per batch
    assert scpb * super_ == cpb and cpb * P == s
    assert d % 1024 == 0 or d <= 1024

    f32 = mybir.dt.float32
    f32r = mybir.dt.float32r
    bf16 = mybir.dt.bfloat16
    # xd: dtype of the streamed x / w / e operands; mm views feed the PE
    xd = bf16 if use_bf16 else f32

    nc = bacc.Bacc(trn_type="TRN2", target_bir_lowering=False, debug=False)
    x_d = nc.declare_dram_parameter("x", [b_pc, s, d], f32, isOutput=False)
    w_d = nc.declare_dram_parameter("w_rep", [P, d], xd, isOutput=False)
    bias_d = nc.declare_dram_parameter("bias", [P, b_pc * cpb], f32, isOutput=False)
    ones_d = nc.declare_dram_parameter("ones", [P, 2], xd, isOutput=False)
    out_d = nc.declare_dram_parameter("out", [b_pc, d], f32, isOutput=True)

    def mm(ap):
        # PE-view of an operand: fp32 operands must be fed as float32r
        # (FP22-truncate-on-read) to stream at 1 cycle/row; bf16 is native.
        return ap if use_bf16 else ap.bitcast(f32r)

    n_half = d // 2          # 512 for the real problem (PSUM fp32 matmul max)
    assert n_half <= 512

    with tile.TileContext(nc) as tc:
        with (
            tc.tile_pool(name="xpool", bufs=x_bufs) as xpool,
            tc.tile_pool(name="ypool", bufs=3) as ypool,
            tc.tile_pool(name="consts", bufs=1) as consts,
            tc.tile_pool(name="small", bufs=8) as small,
            tc.tile_pool(name="outp", bufs=2) as outp,
            tc.tile_pool(name="psum", bufs=2, space="PSUM") as psum_pool,
        ):
            w_sb = consts.tile([P, d], xd)
            nc.sync.dma_start(out=w_sb, in_=w_d[:])
            bias_sb = consts.tile([P, b_pc * cpb], f32)
            nc.sync.dma_start(out=bias_sb, in_=bias_d[:])
            ones_sb = consts.tile([P, 2], xd)
            nc.sync.dma_start(out=mm(ones_sb), in_=mm(ones_d[:]))

            for b in range(b_pc):
                acc0 = psum_pool.tile([1, n_half], f32, tag="acc0")
                acc1 = psum_pool.tile([1, n_half], f32, tag="acc1")
                lps = psum_pool.tile([1, 2], f32, tag="l")
                for sc in range(scpb):
                    xt = xpool.tile([P, super_, d], xd, tag="xt")
                    src = x_d[b, sc * super_ * P : (sc + 1) * super_ * P, :].rearrange(
                        "(j p) d -> p j d", p=P
                    )
                    if use_bf16:
                        # SWDGE casts fp32 -> bf16 inline; HBM read traffic
                        # is unchanged, SBUF tile halves, and the DVE
                        # multiply gets the 2x bf16 perf mode.
                        nc.gpsimd.dma_start(out=xt, in_=src)
                    else:
                        # Write through an f32r-typed AP: the fp32r matmuls
                        # below require their producer to emit fp32r (PE
                        # truncates to FP22 on read; bits are plain fp32).
                        nc.sync.dma_start(out=mm(xt), in_=mm(src))
                    # scores for all super_ chunks of this superchunk,
                    # one column each; exp'd in a single ACT op.
                    scores = small.tile([P, super_], f32, tag="scores")
                    for j in range(super_):
                        y = ypool.tile([P, d], xd, tag="y")
                        nc.vector.tensor_mul(y, xt[:, j, :], w_sb)
                        nc.scalar.activation(
                            y,
                            y,
                            mybir.ActivationFunctionType.Copy,
                            accum_out=scores[:, j : j + 1],
                        )
                    col0 = b * cpb + sc * super_
                    nc.vector.tensor_add(
                        scores, scores, bias_sb[:, col0 : col0 + super_]
                    )
                    e = small.tile([P, super_], xd, tag="e")
                    er = mm(e)
                    nc.scalar.activation(
                        er, scores, mybir.ActivationFunctionType.Exp
                    )
                    for j in range(super_):
                        c = sc * super_ + j
                        first = c == 0
                        last = c == cpb - 1
                        ej = er[:, j : j + 1]
                        nc.tensor.matmul(
                            acc0,
                            ej,
                            mm(xt[:, j, :n_half]),
                            start=first,
                            stop=last,
                        )
                        nc.tensor.matmul(
                            acc1,
                            ej,
                            mm(xt[:, j, n_half:]),
                            start=first,
                            stop=last,
                        )
                        nc.tensor.matmul(
                            lps,
                            ej,
                            mm(ones_sb),
                            start=first,
                            stop=last,
                        )
                linv = small.tile([1, 1], f32, tag="linv")
                nc.vector.reciprocal(linv, lps[:, 0:1])
                ob = outp.tile([1, d], f32, tag="ob")
                nc.vector.tensor_scalar_mul(ob[:, :n_half], acc0, linv)
                nc.vector.tensor_scalar_mul(ob[:, n_half:], acc1, linv)
                nc.sync.dma_start(out=out_d[b : b + 1, :], in_=ob)
    nc.compile()
    return nc


def make_in_maps(x, padding_mask, w, b_pc=B_PC, s=S, d=D, n_cores=N_CORES,
                 use_bf16=False):
    """Shard inputs and build per-core host-side tensors."""
    x = np.asarray(x, dtype=np.float32)
    padding_mask = np.asarray(padding_mask)
    w = np.asarray(w, dtype=np.float32)
    cpb = s // P
    bias = np.where(padding_mask != 0, np.float32(0.0), np.float32(NEG_BIAS))
    bias = bias.astype(np.float32)
    w_rep = np.ascontiguousarray(np.broadcast_to(w.reshape(1, d), (P, d)))
    if use_bf16:
        import ml_dtypes
        w_rep = w_rep.astype(ml_dtypes.bfloat16)
    in_maps = []
    for core in range(n_cores):
        xc = np.ascontiguousarray(x[core * b_pc : (core + 1) * b_pc])
        bc = bias[core * b_pc : (core + 1) * b_pc]  # [b_pc, s]
        # bias_sb[p, b*cpb + c] = bias for row s = c*128 + p of batch b
        bc = np.ascontiguousarray(
            bc.reshape(b_pc, cpb, P).transpose(2, 0, 1).reshape(P, b_pc * cpb)
        )
        ones = np.ones((P, 2), dtype=np.float32)
        if use_bf16:
            import ml_dtypes
            ones = ones.astype(ml_dtypes.bfloat16)
        in_maps.append({"x": xc, "w_rep": w_rep, "bias": bc, "ones": ones})
    return in_maps


# ---------------------------------------------------------------------------
# Packed variant: host packs the unmasked rows of each batch densely (a pure
# layout permutation computed from padding_mask), so the device streams only
# ~kept rows (+pad to a uniform chunk cap) instead of all S rows. Masked rows
# contribute exactly 0 to both softmax numerator and denominator, so dropping
# them is exact. Padding rows are zero-filled and biased to -30000 -> e = 0.
# ---------------------------------------------------------------------------


def _groups(cap_chunks, super_):
    """Split cap_chunks into DMA groups of super_ chunks plus a tail."""
    gs = [super_] * (cap_chunks // super_)
    if cap_chunks % super_:
        gs.append(cap_chunks % super_)
    return gs


def build_bass_packed(b_pc=B_PC, cap_chunks=17, d=D, super_=4, x_bufs=6):
    """Single-core program over host-packed x of [b_pc, cap_chunks*128, d].

    Each DMA group loads g chunks as one [P, g*d] tile with layout
    "(p j) d -> p (j d)": partition p holds rows r = c0*P + p*g + j, i.e. a
    single contiguous g*d*4-byte descriptor per partition.
    """
    import concourse.bacc as bacc
    import concourse.tile as tile
    from concourse import mybir

    f32 = mybir.dt.float32
    f32r = mybir.dt.float32r
    n_half = d // 2
    assert n_half <= 512
    gs = _groups(cap_chunks, super_)
    s_packed = cap_chunks * P

    nc = bacc.Bacc(trn_type="TRN2", target_bir_lowering=False, debug=False)
    x_d = nc.declare_dram_parameter("xp", [b_pc, s_packed, d], f32, isOutput=False)
    w_d = nc.declare_dram_parameter("w_rep", [P, d], f32, isOutput=False)
    bias_d = nc.declare_dram_parameter("bias", [P, b_pc * cap_chunks], f32,
                                       isOutput=False)
    ones_d = nc.declare_dram_parameter("ones", [P, 2], f32, isOutput=False)
    out_d = nc.declare_dram_parameter("out", [b_pc, d], f32, isOutput=True)

    def mm(ap):
        return ap.bitcast(f32r)

    with tile.TileContext(nc) as tc:
        with (
            tc.tile_pool(name="xpool", bufs=x_bufs) as xpool,
            tc.tile_pool(name="ypool", bufs=3) as ypool,
            tc.tile_pool(name="consts", bufs=1) as consts,
            tc.tile_pool(name="small", bufs=8) as small,
            tc.tile_pool(name="outp", bufs=2) as outp,
            tc.tile_pool(name="psum", bufs=2, space="PSUM") as psum_pool,
        ):
            w_sb = consts.tile([P, d], f32)
            nc.sync.dma_start(out=w_sb, in_=w_d[:])
            bias_sb = consts.tile([P, b_pc * cap_chunks], f32)
            nc.sync.dma_start(out=bias_sb, in_=bias_d[:])
            ones_sb = consts.tile([P, 2], f32)
            nc.sync.dma_start(out=mm(ones_sb), in_=mm(ones_d[:]))

            for b in range(b_pc):
                acc0 = psum_pool.tile([1, n_half], f32, tag="acc0")
                acc1 = psum_pool.tile([1, n_half], f32, tag="acc1")
                lps = psum_pool.tile([1, 2], f32, tag="l")
                c0 = 0
                for g in gs:
                    xt = xpool.tile([P, g * d], f32, tag=f"xt{g}")
                    src = x_d[b, c0 * P : (c0 + g) * P, :].rearrange(
                        "(p j) d -> p (j d)", j=g
                    )
                    nc.sync.dma_start(out=mm(xt), in_=mm(src))
                    scores = small.tile([P, g], f32, tag=f"scores{g}")
                    for j in range(g):
                        y = ypool.tile([P, d], f32, tag="y")
                        nc.vector.tensor_mul(y, xt[:, j * d : (j + 1) * d], w_sb)
                        nc.scalar.activation(
                            y,
                            y,
                            mybir.ActivationFunctionType.Copy,
                            accum_out=scores[:, j : j + 1],
                        )
                    col0 = b * cap_chunks + c0
                    nc.vector.tensor_add(
                        scores, scores, bias_sb[:, col0 : col0 + g]
                    )
                    e = small.tile([P, g], f32, tag=f"e{g}")
                    er = mm(e)
                    nc.scalar.activation(
                        er, scores, mybir.ActivationFunctionType.Exp
                    )
                    for j in range(g):
                        c = c0 + j
                        first = c == 0
                        last = c == cap_chunks - 1
                        ej = er[:, j : j + 1]
                        nc.tensor.matmul(
                            acc0, ej, mm(xt[:, j * d : j * d + n_half]),
                            start=first, stop=last,
                        )
                        nc.tensor.matmul(
                            acc1, ej, mm(xt[:, j * d + n_half : (j + 1) * d]),
                            start=first, stop=last,
                        )
                        nc.tensor.matmul(
                            lps, ej, mm(ones_sb), start=first, stop=last,
                        )
                    c0 += g
                linv = small.tile([1, 1], f32, tag="linv")
                nc.vector.reciprocal(linv, lps[:, 0:1])
                ob = outp.tile([1, d], f32, tag="ob")
                nc.vector.tensor_scalar_mul(ob[:, :n_half], acc0, linv)
                nc.vector.tensor_scalar_mul(ob[:, n_half:], acc1, linv)
                nc.sync.dma_start(out=out_d[b : b + 1, :], in_=ob)
    nc.compile()
    return nc


def make_in_maps_packed(x, padding_mask, w, b_pc=B_PC, s=S, d=D,
                        n_cores=N_CORES, super_=4):
    """Host prep: pack kept rows densely per batch; returns (in_maps, cap)."""
    x = np.asarray(x, dtype=np.float32)
    padding_mask = np.asarray(padding_mask)
    w = np.asarray(w, dtype=np.float32)
    keeps = [np.flatnonzero(padding_mask[b]) for b in range(n_cores * b_pc)]
    max_kept = max(1, max(len(k) for k in keeps))
    cap = -(-max_kept // P)          # chunks, uniform across all batches
    cap = min(cap, s // P)
    gs = _groups(cap, super_)
    w_rep = np.ascontiguousarray(np.broadcast_to(w.reshape(1, d), (P, d)))
    ones = np.ones((P, 2), dtype=np.float32)
    p_ar = np.arange(P)
    in_maps = []
    for core in range(n_cores):
        xp = np.zeros((b_pc, cap * P, d), dtype=np.float32)
        bias_cols = np.empty((P, b_pc * cap), dtype=np.float32)
        for bi in range(b_pc):
            keep = keeps[core * b_pc + bi]
            k = len(keep)
            xp[bi, :k] = x[core * b_pc + bi, keep]
            c0 = 0
            for g in gs:
                for j in range(g):
                    r = c0 * P + p_ar * g + j
                    bias_cols[:, bi * cap + c0 + j] = np.where(
                        r < k, np.float32(0.0), np.float32(NEG_BIAS)
                    )
                c0 += g
        in_maps.append({
            "xp": xp, "w_rep": w_rep,
            "bias": np.ascontiguousarray(bias_cols), "ones": ones,
        })
    return in_maps, cap


_NC_CACHE = {}


def _get_nc():
    if "nc" not in _NC_CACHE:
        _NC_CACHE["nc"] = build_bass()
    return _NC_CACHE["nc"]


PACKED_SUPER = 4


def _get_nc_packed(cap, super_=PACKED_SUPER):
    key = ("packed", cap, super_)
    if key not in _NC_CACHE:
        _NC_CACHE[key] = build_bass_packed(
            cap_chunks=cap, super_=super_
        )
    return _NC_CACHE[key]


def kernel(x, padding_mask, w):
    from concourse.bass_utils import run_bass_kernel_spmd

    in_maps, cap = make_in_maps_packed(x, padding_mask, w, super_=PACKED_SUPER)
    nc = _get_nc_packed(cap)
    res = run_bass_kernel_spmd(nc, in_maps, list(range(N_CORES)))
    outs = [res.results[c]["out"] for c in range(N_CORES)]
    return np.concatenate(outs, axis=0).astype(np.float32)


# ---------------------------------------------------------------------------
# Mask-gather variant: only unmasked rows are loaded (masked rows contribute
# exactly 0 to softmax numerator and denominator). Row indices come from the
# runtime mask (host-computed, passed as an int16 input) via dma_gather.
# ---------------------------------------------------------------------------

def build_bass_gather(b_pc=B_PC, s=S, d=D, cap_chunks=20, half_chunks=10,
                      x_bufs=4):
    import concourse.bacc as bacc
    import concourse.tile as tile
    from concourse import mybir

    cap = cap_chunks * P
    halves = cap_chunks // half_chunks
    assert halves * half_chunks == cap_chunks
    f32 = mybir.dt.float32
    f32r = mybir.dt.float32r
    i16 = mybir.dt.int16
    n_half = d // 2
    nidx_half = half_chunks * P           # rows per gather call

    nc = bacc.Bacc(trn_type="TRN2", target_bir_lowering=False, debug=False)
    x_d = nc.declare_dram_parameter("x", [b_pc, s, d], f32, isOutput=False)
    w_d = nc.declare_dram_parameter("w_rep", [P, d], f32, isOutput=False)
    bias_d = nc.declare_dram_parameter("bias", [P, b_pc * cap_chunks], f32,
                                       isOutput=False)
    idx_d = nc.declare_dram_parameter(
        "idx", [P, b_pc * halves * (nidx_half // 16)], i16, isOutput=False)
    ones_d = nc.declare_dram_parameter("ones", [P, 2], f32, isOutput=False)
    out_d = nc.declare_dram_parameter("out", [b_pc, d], f32, isOutput=True)

    with tile.TileContext(nc) as tc:
        with (
            tc.tile_pool(name="xpool", bufs=x_bufs) as xpool,
            tc.tile_pool(name="ypool", bufs=3) as ypool,
            tc.tile_pool(name="consts", bufs=1) as consts,
            tc.tile_pool(name="small", bufs=8) as small,
            tc.tile_pool(name="outp", bufs=2) as outp,
            tc.tile_pool(name="psum", bufs=2, space="PSUM") as psum_pool,
        ):
            w_sb = consts.tile([P, d], f32)
            nc.sync.dma_start(out=w_sb, in_=w_d[:])
            bias_sb = consts.tile([P, b_pc * cap_chunks], f32)
            nc.sync.dma_start(out=bias_sb, in_=bias_d[:])
            idx_sb = consts.tile([P, b_pc * halves * (nidx_half // 16)], i16)
            nc.sync.dma_start(out=idx_sb, in_=idx_d[:])
            ones_sb = consts.tile([P, 2], f32)
            nc.sync.dma_start(out=ones_sb.bitcast(f32r), in_=ones_d[:].bitcast(f32r))

            icols = nidx_half // 16
            for b in range(b_pc):
                acc0 = psum_pool.tile([1, n_half], f32, tag="acc0")
                acc1 = psum_pool.tile([1, n_half], f32, tag="acc1")
                lps = psum_pool.tile([1, 2], f32, tag="l")
                for h in range(halves):
                    xt = xpool.tile([P, half_chunks, d], f32, tag="xt")
                    islice = idx_sb[:, (b * halves + h) * icols
                                    : (b * halves + h + 1) * icols]
                    nc.gpsimd.dma_gather(
                        out_ap=xt.bitcast(f32r),
                        in_ap=x_d[b].bitcast(f32r),
                        idxs_ap=islice,
                        num_idxs=nidx_half,
                        num_idxs_reg=nidx_half,
                        elem_size=d,
                    )
                    scores = small.tile([P, half_chunks], f32, tag="scores")
                    for j in range(half_chunks):
                        y = ypool.tile([P, d], f32, tag="y")
                        nc.vector.tensor_mul(y, xt[:, j, :], w_sb)
                        nc.scalar.activation(
                            y, y, mybir.ActivationFunctionType.Copy,
                            accum_out=scores[:, j : j + 1],
                        )
                    col0 = b * cap_chunks + h * half_chunks
                    nc.vector.tensor_add(
                        scores, scores, bias_sb[:, col0 : col0 + half_chunks]
                    )
                    e = small.tile([P, half_chunks], f32, tag="e")
                    er = e.bitcast(f32r)
                    nc.scalar.activation(
                        er, scores, mybir.ActivationFunctionType.Exp
                    )
                    for j in range(half_chunks):
                        c = h * half_chunks + j
                        first = c == 0
                        last = c == cap_chunks - 1
                        ej = er[:, j : j + 1]
                        nc.tensor.matmul(acc0, ej, xt[:, j, :n_half].bitcast(f32r),
                                         start=first, stop=last)
                        nc.tensor.matmul(acc1, ej, xt[:, j, n_half:].bitcast(f32r),
                                         start=first, stop=last)
                        nc.tensor.matmul(lps, ej, ones_sb.bitcast(f32r),
                                         start=first, stop=last)
                linv = small.tile([1, 1], f32, tag="linv")
                nc.vector.reciprocal(linv, lps[:, 0:1])
                ob = outp.tile([1, d], f32, tag="ob")
                nc.vector.tensor_scalar_mul(ob[:, :n_half], acc0, linv)
                nc.vector.tensor_scalar_mul(ob[:, n_half:], acc1, linv)
                nc.sync.dma_start(out=out_d[b : b + 1, :], in_=ob)
    nc.compile()
    return nc


def make_in_maps_gather(x, padding_mask, w, b_pc=B_PC, s=S, d=D,
                        n_cores=N_CORES, cap_chunks=20, half_chunks=10):
    """Host prep for the gather variant. Returns None if any batch has more
    unmasked rows than cap_chunks*128 (caller falls back to dense)."""
    x = np.asarray(x, dtype=np.float32)
    padding_mask = np.asarray(padding_mask)
    w = np.asarray(w, dtype=np.float32)
    cap = cap_chunks * P
    halves = cap_chunks // half_chunks
    nidx_half = half_chunks * P
    icols = nidx_half // 16
    w_rep = np.ascontiguousarray(np.broadcast_to(w.reshape(1, d), (P, d)))
    in_maps = []
    for core in range(n_cores):
        xc = np.ascontiguousarray(x[core * b_pc : (core + 1) * b_pc])
        mc = padding_mask[core * b_pc : (core + 1) * b_pc]
        bias_cols = np.zeros((P, b_pc * cap_chunks), dtype=np.float32)
        idx_cols = np.zeros((16, b_pc * halves * icols), dtype=np.int16)
        for b in range(b_pc):
            keep = np.where(mc[b] != 0)[0]
            if len(keep) > cap:
                return None
            idxs = np.zeros(cap, dtype=np.int16)
            idxs[: len(keep)] = keep.astype(np.int16)
            biasvec = np.zeros(cap, dtype=np.float32)
            biasvec[len(keep):] = NEG_BIAS
            bias_cols[:, b * cap_chunks : (b + 1) * cap_chunks] = (
                biasvec.reshape(cap_chunks, P).T
            )
            for h in range(halves):
                part = idxs[h * nidx_half : (h + 1) * nidx_half]
                # index k -> partition k%16, column k//16
                idx_cols[:, (b * halves + h) * icols
                         : (b * halves + h + 1) * icols] = (
                    part.reshape(icols, 16).T
                )
        idx_full = np.ascontiguousarray(np.tile(idx_cols, (8, 1)))
        ones = np.ones((P, 2), dtype=np.float32)
        in_maps.append({
            "x": xc, "w_rep": w_rep, "bias": np.ascontiguousarray(bias_cols),
            "idx": idx_full, "ones": ones,
        })
    return in_maps


def build_bass_gather2(b_pc=B_PC, s=S, d=D, cap_chunks=20, x_bufs=6):
    """Mask-gather via per-chunk indirect_dma_start (plain InstDMACopy with
    dynamic AP — no GpSimd library overlay, unlike dma_gather)."""
    import concourse.bacc as bacc
    import concourse.bass as bass
    import concourse.tile as tile
    from concourse import mybir

    f32 = mybir.dt.float32
    f32r = mybir.dt.float32r
    i32 = mybir.dt.int32
    n_half = d // 2

    nc = bacc.Bacc(trn_type="TRN2", target_bir_lowering=False, debug=False)
    x_d = nc.declare_dram_parameter("x", [b_pc, s, d], f32, isOutput=False)
    w_d = nc.declare_dram_parameter("w_rep", [P, d], f32, isOutput=False)
    bias_d = nc.declare_dram_parameter("bias", [P, b_pc * cap_chunks], f32,
                                       isOutput=False)
    idx_d = nc.declare_dram_parameter("idx", [P, b_pc * cap_chunks], i32,
                                      isOutput=False)
    ones_d = nc.declare_dram_parameter("ones", [P, 2], f32, isOutput=False)
    out_d = nc.declare_dram_parameter("out", [b_pc, d], f32, isOutput=True)

    x_flat = x_d[:].rearrange("b s d -> (b s) d").bitcast(f32r)
    with tile.TileContext(nc) as tc:
        with (
            tc.tile_pool(name="xpool", bufs=x_bufs) as xpool,
            tc.tile_pool(name="ypool", bufs=3) as ypool,
            tc.tile_pool(name="consts", bufs=1) as consts,
            tc.tile_pool(name="small", bufs=8) as small,
            tc.tile_pool(name="outp", bufs=2) as outp,
            tc.tile_pool(name="psum", bufs=2, space="PSUM") as psum_pool,
        ):
            w_sb = consts.tile([P, d], f32)
            nc.sync.dma_start(out=w_sb, in_=w_d[:])
            bias_sb = consts.tile([P, b_pc * cap_chunks], f32)
            nc.sync.dma_start(out=bias_sb, in_=bias_d[:])
            idx_sb = consts.tile([P, b_pc * cap_chunks], i32)
            nc.sync.dma_start(out=idx_sb, in_=idx_d[:])
            ones_sb = consts.tile([P, 2], f32)
            nc.sync.dma_start(out=ones_sb.bitcast(f32r), in_=ones_d[:].bitcast(f32r))

            for b in range(b_pc):
                acc0 = psum_pool.tile([1, n_half], f32, tag="acc0")
                acc1 = psum_pool.tile([1, n_half], f32, tag="acc1")
                lps = psum_pool.tile([1, 2], f32, tag="l")
                for c in range(cap_chunks):
                    col = b * cap_chunks + c
                    xt = xpool.tile([P, d], f32, tag="xt")
                    nc.gpsimd.indirect_dma_start(
                        out=xt.bitcast(f32r),
                        out_offset=None,
                        in_=x_flat,
                        in_offset=bass.IndirectOffsetOnAxis(
                            ap=idx_sb[:, col : col + 1], axis=0
                        ),
                    )
                    y = ypool.tile([P, d], f32, tag="y")
                    nc.vector.tensor_mul(y, xt, w_sb)
                    scores = small.tile([P, 1], f32, tag="scores")
                    nc.scalar.activation(
                        y, y, mybir.ActivationFunctionType.Copy,
                        accum_out=scores,
                    )
                    e = small.tile([P, 1], f32, tag="e")
                    er = e.bitcast(f32r)
                    nc.scalar.activation(
                        er, scores, mybir.ActivationFunctionType.Exp,
                        bias=bias_sb[:, col : col + 1],
                    )
                    first = c == 0
                    last = c == cap_chunks - 1
                    nc.tensor.matmul(acc0, er, xt[:, :n_half].bitcast(f32r),
                                     start=first, stop=last)
                    nc.tensor.matmul(acc1, er, xt[:, n_half:].bitcast(f32r),
                                     start=first, stop=last)
                    nc.tensor.matmul(lps, er, ones_sb.bitcast(f32r),
                                     start=first, stop=last)
                linv = small.tile([1, 1], f32, tag="linv")
                nc.vector.reciprocal(linv, lps[:, 0:1])
                ob = outp.tile([1, d], f32, tag="ob")
                nc.vector.tensor_scalar_mul(ob[:, :n_half], acc0, linv)
                nc.vector.tensor_scalar_mul(ob[:, n_half:], acc1, linv)
                nc.sync.dma_start(out=out_d[b : b + 1, :], in_=ob)
    nc.compile()
    return nc


def make_in_maps_gather2(x, padding_mask, w, b_pc=B_PC, s=S, d=D,
                         n_cores=N_CORES, cap_chunks=20):
    x = np.asarray(x, dtype=np.float32)
    padding_mask = np.asarray(padding_mask)
    w = np.asarray(w, dtype=np.float32)
    cap = cap_chunks * P
    w_rep = np.ascontiguousarray(np.broadcast_to(w.reshape(1, d), (P, d)))
    in_maps = []
    for core in range(n_cores):
        xc = np.ascontiguousarray(x[core * b_pc : (core + 1) * b_pc])
        mc = padding_mask[core * b_pc : (core + 1) * b_pc]
        bias_cols = np.zeros((P, b_pc * cap_chunks), dtype=np.float32)
        idx_cols = np.zeros((P, b_pc * cap_chunks), dtype=np.int32)
        for b in range(b_pc):
            keep = np.where(mc[b] != 0)[0]
            if len(keep) > cap:
                return None
            idxs = np.full(cap, b * s, dtype=np.int32)
            idxs[: len(keep)] = keep + b * s
            biasvec = np.zeros(cap, dtype=np.float32)
            biasvec[len(keep):] = NEG_BIAS
            sl = slice(b * cap_chunks, (b + 1) * cap_chunks)
            bias_cols[:, sl] = biasvec.reshape(cap_chunks, P).T
            idx_cols[:, sl] = idxs.reshape(cap_chunks, P).T
        in_maps.append({
            "x": xc, "w_rep": w_rep,
            "bias": np.ascontiguousarray(bias_cols),
            "idx": np.ascontiguousarray(idx_cols),
            "ones": np.ones((P, 2), dtype=np.float32),
        })
    return in_maps

